# revision 11
# baseline (speedup 1.0000x reference)
"""Trainium2 Bass kernel for:
    logits4 = einsum('bic,bjc->bijc', Q, K) + bias      # [B,I,J,C]
    output  = sigmoid(logits4).mean(axis=-2)            # [B,I,C]
    attention_logits = einsum('bic,bjc->bij', Q, K)     # [B,I,J]
    return (output, attention_logits)

B,I,J,C = 4,512,512,512. Runs SPMD on 8 NeuronCores: core k handles
(b = k//2, h = k%2) with the sigmoid-mean part sharded over C-halves
(all I), and the attention-logits matmul sharded over I-halves (all C).

Per-core dataflow:
  - TensorE builds the biased outer-product tiles
    P[i,j] = Q[i,c]*K[j,c] + bias[c] with contraction-dim-2 matmuls:
    lhsT = (Q^T row c, ones) x i-block, rhs = (K^T row c, bias[c]*ones).
    Operand pairs live on SBUF partitions 0-1 (PE requires base partition
    in {0,32,64,96}), packed c-major along the free dim and streamed from
    DRAM in 8-c chunks. 4 i-blocks per c -> PSUM [128, 2048] (4 banks),
    double buffered (8 banks = all of PSUM).
  - ScalarE applies sigmoid on the whole [128, 2048] PSUM group
    (bf16 out to SBUF), one instruction per c.  <- the bottleneck engine
  - VectorE reduces over j with one tensor_tensor_reduce per (c, i-block):
    out = (sig[:, :256] + sig[:, 256:]) * (1/J), accum_out = sum -> mean.
  - attention_logits: plain QK^T matmuls at the end, reusing PSUM slots.
"""
import os

if "JAX_PLATFORMS" in os.environ and "axon" not in os.environ["JAX_PLATFORMS"]:
    # the bass kernel executes through the axon PJRT backend
    os.environ["JAX_PLATFORMS"] = ""

import numpy as np
import ml_dtypes

import concourse.bacc as bacc
import concourse.mybir as mybir
from concourse import tile
from concourse.bass_utils import run_bass_kernel_spmd

B, I, J, C = 4, 512, 512, 512
NCORES = 8
CH = C // 2          # c-half per core
IH = I // 2          # i-half per core
NIB = I // 128       # i-blocks (4)
CHUNK = 8            # c's per staged operand tile

BF16 = mybir.dt.bfloat16
F32 = mybir.dt.float32
ADD = mybir.AluOpType.add

REDUCE_MODE = "std"   # "std" (DVE tensor_reduce), "act_accum" (ACT accum_out),
                      # or "ttr" (DVE tensor_tensor_reduce; faults on HW)


def build_nc():
    nc = bacc.Bacc("TRN2", target_bir_lowering=False, debug=False, num_devices=NCORES)

    # qp: [0, c*I + i] = Q^T[c, i], [1, :] = 1.0
    # kp: [0, c*J + j] = K^T[c, j], [1, c*J + j] = bias[c]
    qp = nc.dram_tensor("qp", [2, CH * I], BF16, kind="ExternalInput")
    kp = nc.dram_tensor("kp", [2, CH * J], BF16, kind="ExternalInput")
    qt = nc.dram_tensor("qt", [C, IH], BF16, kind="ExternalInput")   # Q^T, i-half
    kt = nc.dram_tensor("kt", [C, J], BF16, kind="ExternalInput")    # K^T, full
    out_mean = nc.dram_tensor("out_mean", [I, CH], F32, kind="ExternalOutput")
    out_logits = nc.dram_tensor("out_logits", [IH, J], F32, kind="ExternalOutput")

    with tile.TileContext(nc) as tc:
        with (
            tc.tile_pool(name="sb", bufs=1) as sb,
            tc.tile_pool(name="st", bufs=3) as st,
            tc.tile_pool(name="mp", bufs=2, space="PSUM") as mp,
            tc.tile_pool(name="sg", bufs=3) as sg,
        ):
            qt_t = []
            kt_t = []
            for t in range(C // 128):
                a = sb.tile([128, IH], BF16, tag=f"qt{t}")
                nc.sync.dma_start(a[:], qt[128 * t : 128 * (t + 1), :])
                qt_t.append(a)
                b = sb.tile([128, J], BF16, tag=f"kt{t}")
                nc.sync.dma_start(b[:], kt[128 * t : 128 * (t + 1), :])
                kt_t.append(b)

            # means land here: stage[p, ib*CH + cc] = mean[ib*128+p, cc]
            stage = sb.tile([128, NIB * CH], F32, tag="stage")

            for chunk in range(CH // CHUNK):
                c0 = chunk * CHUNK
                qs = st.tile([2, CHUNK * I], BF16, tag="qs")
                nc.sync.dma_start(qs[:], qp[:, c0 * I : (c0 + CHUNK) * I])
                ks = st.tile([2, CHUNK * J], BF16, tag="ks")
                nc.sync.dma_start(ks[:], kp[:, c0 * J : (c0 + CHUNK) * J])
                for m in range(CHUNK):
                    c = c0 + m
                    ps = mp.tile([128, NIB * J], F32, tag="ps")
                    for ib in range(NIB):
                        nc.tensor.matmul(
                            ps[:, ib * J : (ib + 1) * J],
                            qs[:, m * I + ib * 128 : m * I + (ib + 1) * 128],
                            ks[:, m * J : (m + 1) * J],
                            start=True,
                            stop=True,
                        )
                    # ACT reads must stay within ONE PSUM bank (multi-bank
                    # APs hang the engine) -> one ACTIVATE per i-block.
                    for ib in range(NIB):
                        sig = sg.tile([128, J], BF16, tag="sig")
                        if REDUCE_MODE == "act_accum":
                            nc.scalar.activation(
                                sig[:],
                                ps[:, ib * J : (ib + 1) * J],
                                mybir.ActivationFunctionType.Sigmoid,
                                accum_out=stage[:, ib * CH + c : ib * CH + c + 1],
                            )
                            continue
                        nc.scalar.activation(
                            sig[:],
                            ps[:, ib * J : (ib + 1) * J],
                            mybir.ActivationFunctionType.Sigmoid,
                        )
                        if REDUCE_MODE == "ttr":
                            scr = sg.tile([128, J // 2], BF16, tag="scr")
                            nc.vector.tensor_tensor_reduce(
                                out=scr[:],
                                in0=sig[:, : J // 2],
                                in1=sig[:, J // 2 :],
                                scale=1.0 / J,
                                scalar=0.0,
                                op0=ADD,
                                op1=ADD,
                                accum_out=stage[:, ib * CH + c : ib * CH + c + 1],
                            )
                        else:
                            nc.vector.tensor_reduce(
                                stage[:, ib * CH + c : ib * CH + c + 1],
                                sig[:],
                                axis=mybir.AxisListType.X,
                                op=ADD,
                            )

            if REDUCE_MODE in ("std", "act_accum"):
                nc.vector.tensor_scalar_mul(stage[:], stage[:], 1.0 / J)
            for ib in range(NIB):
                nc.sync.dma_start(
                    out_mean[ib * 128 : (ib + 1) * 128, :],
                    stage[:, ib * CH : (ib + 1) * CH],
                )

            # attention logits (end of PE stream; reuses a PSUM slot)
            ps_lg = mp.tile([128, NIB * J], F32, tag="ps")
            for it in range(IH // 128):
                for cb in range(C // 128):
                    nc.tensor.matmul(
                        ps_lg[:, it * J : (it + 1) * J],
                        qt_t[cb][:, it * 128 : (it + 1) * 128],
                        kt_t[cb][:],
                        start=(cb == 0),
                        stop=(cb == C // 128 - 1),
                    )
            for it in range(IH // 128):
                lg = sb.tile([128, J], F32, tag=f"lg{it}")
                nc.vector.tensor_copy(lg[:], ps_lg[:, it * J : (it + 1) * J])
                nc.sync.dma_start(out_logits[it * 128 : (it + 1) * 128, :], lg[:])

    nc.compile()
    return nc


def make_in_maps(Q, K, bias):
    Q = np.asarray(Q, dtype=np.float32)
    K = np.asarray(K, dtype=np.float32)
    bias = np.asarray(bias, dtype=np.float32)
    in_maps = []
    for core in range(NCORES):
        b, h = core // 2, core % 2
        cs = slice(h * CH, (h + 1) * CH)
        QT = Q[b].T.astype(ml_dtypes.bfloat16)  # [C, I]
        KT = K[b].T.astype(ml_dtypes.bfloat16)  # [C, J]
        qp = np.empty((2, CH, I), dtype=ml_dtypes.bfloat16)
        qp[0] = QT[cs]
        qp[1] = np.float32(1.0)
        kp = np.empty((2, CH, J), dtype=ml_dtypes.bfloat16)
        kp[0] = KT[cs]
        kp[1] = bias[cs].astype(ml_dtypes.bfloat16)[:, None]
        in_maps.append(
            {
                "qp": qp.reshape(2, CH * I),
                "kp": kp.reshape(2, CH * J),
                "qt": np.ascontiguousarray(QT[:, h * IH : (h + 1) * IH]),
                "kt": np.ascontiguousarray(KT),
            }
        )
    return in_maps


def assemble(results):
    output = np.empty((B, I, C), dtype=np.float32)
    attention_logits = np.empty((B, I, J), dtype=np.float32)
    for core in range(NCORES):
        b, h = core // 2, core % 2
        output[b, :, h * CH : (h + 1) * CH] = results[core]["out_mean"]
        attention_logits[b, h * IH : (h + 1) * IH, :] = results[core]["out_logits"]
    return output, attention_logits


def build_null_nc():
    """Minimal kernel used by test.py to measure dispatch overhead."""
    nc = bacc.Bacc("TRN2", target_bir_lowering=False, debug=False, num_devices=NCORES)
    x = nc.dram_tensor("x", [8, 8], F32, kind="ExternalInput")
    y = nc.dram_tensor("y", [8, 8], F32, kind="ExternalOutput")
    with tile.TileContext(nc) as tc:
        with tc.tile_pool(name="p", bufs=1) as pool:
            t = pool.tile([8, 8], F32)
            nc.sync.dma_start(t[:], x[:])
            nc.sync.dma_start(y[:], t[:])
    nc.compile()
    return nc


_NC = None


def get_nc():
    global _NC
    if _NC is None:
        _NC = build_nc()
    return _NC


def run(Q, K, bias, **kwargs):
    nc = get_nc()
    res = run_bass_kernel_spmd(
        nc, make_in_maps(Q, K, bias), core_ids=list(range(NCORES)), **kwargs
    )
    return res


def kernel(Q, K, bias):
    res = run(Q, K, bias)
    return assemble(res.results)
